# revision 7
# baseline (speedup 1.0000x reference)
"""MultiHeadCrossAttention Trainium2 kernel v2 (8-core data-parallel).

Shapes (hardcoded): B=16, SQ=SE=1024, C_IN=C_ENC=256, DK=DV=64, H=8.
Sharding: batch across 8 cores (2 batches/core).

v2 design vs baseline:
- q/x pre-transposed to [C, S] on host -> no PE transposes, no evac copies.
- Output projection in [c, s] layout so BN stats/apply are per-partition.
- Softmax denominator via ones-column in V; normalization multiply on DVE
  with accum_out giving per-head channel sums (-> BN mean via tiny matvec).
- Pool engine handles PSUM evacuation of Q/K projections, recip broadcast,
  and sum(p^2); ScalarE does nothing but exp (the critical path).
- Final y transposed back to [s, c] via DMA SBUF->SBUF transpose (bf16).
- Software-pipelined emission: scores(h+1) issued before AV(h).
"""
import sys

sys.path.insert(0, "/opt/trn_rl_repo")

import numpy as np

import concourse.bacc as bacc
import concourse.tile as tile
from concourse import mybir
from concourse.bass_utils import run_bass_kernel_spmd

F32 = mybir.dt.float32
F32R = mybir.dt.float32r
BF16 = mybir.dt.bfloat16

B, SQ, SE = 16, 1024, 1024
C, DK, DV, H = 256, 64, 64, 8
BN_EPS = 1e-5
NEG_SLOPE = 0.01
N_CORES = 8
BL = B // N_CORES
P = 128
NT = SE // P   # 8 key tiles
NST = SQ // P  # 8 query tiles
SCH = 2        # s-chunks of 512 per head
SCW = SQ // SCH


def build_kernel(n_cores=N_CORES, with_collective=True):
    nc = bacc.Bacc("TRN2", target_bir_lowering=False, debug=False,
                   num_devices=n_cores)

    qt_d = nc.declare_dram_parameter("qt", [BL, 2, P, SQ], F32R, isOutput=False)
    xt_d = nc.declare_dram_parameter("xt", [BL, 2, P, SE], F32R, isOutput=False)
    wq_d = nc.declare_dram_parameter("wq", [P, 2, H, DK], F32R, isOutput=False)
    wk_d = nc.declare_dram_parameter("wk", [P, 2, H, DK], F32R, isOutput=False)
    wv_d = nc.declare_dram_parameter("wv", [P, 2, H * DV], F32R, isOutput=False)
    wp_d = nc.declare_dram_parameter("wp", [P, H // 2, C], F32R, isOutput=False)
    gb_d = nc.declare_dram_parameter("gb", [P, 2, 2], F32, isOutput=False)
    y_d = nc.declare_dram_parameter("y", [BL, 2, P, SQ], BF16, isOutput=True)

    with tile.TileContext(nc) as tc:
        with (
            tc.tile_pool(name="const", bufs=1) as const,
            tc.tile_pool(name="qx", bufs=2) as qxp,       # qT/xT inputs
            tc.tile_pool(name="qk", bufs=2) as qkp,       # qhT/khT projections
            tc.tile_pool(name="vh", bufs=2) as vhp,       # vh_aug values
            tc.tile_pool(name="pt", bufs=2) as ptp,       # exp(scores)
            tc.tile_pool(name="ot", bufs=2) as otp,       # normalized attn out
            tc.tile_pool(name="pp", bufs=1) as ppp,       # projected p (both b)
            tc.tile_pool(name="sm", bufs=2) as sm,        # small scratch
            tc.tile_pool(name="yy", bufs=2) as yyp,       # y staging
            tc.tile_pool(name="fin", bufs=1) as fin,
            tc.tile_pool(name="sp_ps", bufs=2, space="PSUM") as sp_ps,   # 2x4KB
            tc.tile_pool(name="av_ps", bufs=1, space="PSUM") as av_ps,   # 1x4KB
            tc.tile_pool(name="mm_ps", bufs=2, space="PSUM") as mm_ps,   # 2x2KB
            tc.tile_pool(name="dram", bufs=1, space="DRAM") as dram,
        ):
            wq_sb = const.tile([P, 2, H, DK], F32R, tag="wq")
            wk_sb = const.tile([P, 2, H, DK], F32R, tag="wk")
            wv_sb = const.tile([P, 2, H * DV], F32R, tag="wv")
            wp_sb = const.tile([P, H // 2, C], F32R, tag="wp")
            gb_sb = const.tile([P, 2, 2], F32, tag="gb")
            # wk issued first, then xT (in prep_start), then wq/qT;
            # wv/wp/gb issued after the input loads.
            nc.sync.dma_start(out=wk_sb, in_=wk_d[:])

            # projected p for both batches, [c, ct, b, s] layout
            p_sb = ppp.tile([P, 2, BL, SQ], BF16, tag="p")
            sig_o = []   # per-batch [128, 8] head-channel sums of oT

            def proj_slab(w_sb, src, dst, m, sc0=0, sc1=SCH):
                for sc in range(sc0, sc1):
                    pj = mm_ps.tile([P, SCW], F32, tag="mm")
                    for k in range(2):
                        nc.tensor.matmul(
                            pj[:],
                            w_sb[:, k, 2 * m:2 * m + 2, :],
                            src[:, k, sc * SCW:(sc + 1) * SCW],
                            start=(k == 0), stop=(k == 1))
                    nc.vector.tensor_copy(
                        dst[:, m, sc * SCW:(sc + 1) * SCW], pj[:])

            preps = {}
            vh_done = {}
            m_done = {}

            def prep_start(b):
                """Load qT/xT; project head-pair 0 only (unblocks scores(0))."""
                xT = qxp.tile([P, 2, SE], F32R, tag="qx")
                qT = qxp.tile([P, 2, SQ], F32R, tag="qx")
                for k in range(2):
                    nc.sync.dma_start(out=xT[:, k, :], in_=xt_d[b, k])
                if b == 0:
                    nc.sync.dma_start(out=wq_sb, in_=wq_d[:])
                for k in range(2):
                    nc.sync.dma_start(out=qT[:, k, :], in_=qt_d[b, k])
                qhT = qkp.tile([P, H // 2, SQ], F32R, tag="qk")
                khT = qkp.tile([P, H // 2, SE], F32R, tag="qk")
                preps[b] = (qT, xT, qhT, khT, None)
                m_done[b] = -1
                proj_slab(wk_sb, xT, khT, 0)
                if b == 0:
                    proj_slab(wq_sb, qT, qhT, 0)
                    m_done[b] = 0

            def vh_alloc(b):
                qT, xT, qhT, khT, _ = preps[b]
                vh_aug = vhp.tile([P, NT, H, DV + 1], BF16, tag="vh")
                nc.vector.memset(vh_aug[:, :, :, DV:DV + 1], 1.0)
                preps[b] = (qT, xT, qhT, khT, vh_aug)

            def vproj(b, t0, t1):
                qT, xT, qhT, khT, vh_aug = preps[b]
                for t in range(t0, t1):
                    pj = mm_ps.tile([P, H * DV], F32, tag="mm")
                    for k in range(2):
                        nc.tensor.matmul(
                            pj[:], xT[:, k, t * P:(t + 1) * P], wv_sb[:, k, :],
                            start=(k == 0), stop=(k == 1))
                    nc.vector.tensor_copy(
                        vh_aug[:, t, :, 0:DV],
                        pj.rearrange("p (h e) -> p h e", h=H))

            def prep_units(b, with_start):
                """Work units (~1.2us PE each) finishing what prep_start began."""
                units = []
                def qslab(m):
                    proj_slab(wq_sb, preps[b][0], preps[b][2], m)
                    m_done[b] = m

                if with_start:
                    units.append(lambda: prep_start(b))
                    units.append(lambda: qslab(0))
                    units.append(lambda: (vh_alloc(b), vproj(b, 0, 4)))
                    units.append(lambda: (vproj(b, 4, 8),
                                          vh_done.__setitem__(b, True)))
                for m in range(1, H // 2):
                    units.append(lambda m=m: proj_slab(
                        wk_sb, preps[b][1], preps[b][3], m))
                    units.append(lambda m=m: qslab(m))
                return units

            def scores(h, qhT, khT, t0=0, t1=NT, sp=None):
                """Raw attention scores for head h: NT x [key 128, 2, 512] PSUM."""
                par = 64 * (h % 2)
                j = h // 2
                if sp is None:
                    sp = []
                for t in range(t0, t1):
                    spt = sp_ps.tile([P, SCH, SCW], F32, tag="sp")
                    for sc in range(SCH):
                        nc.tensor.matmul(
                            spt[:, sc, :],
                            khT[par:par + 64, j, t * P:(t + 1) * P],
                            qhT[par:par + 64, j, sc * SCW:(sc + 1) * SCW],
                            start=True, stop=True)
                    sp.append(spt)
                return sp

            def exp_head(h, sp, pt):
                for t in range(NT):
                    nc.scalar.activation(
                        out=pt[:, t, :],
                        in_=sp[t].rearrange("p a b -> p (a b)"),
                        func=mybir.ActivationFunctionType.Exp,
                        scale=1.0 / np.sqrt(DK).item())

            def av_half(h, pt, vh_aug, av, sc):
                for t in range(NT):
                    nc.tensor.matmul(
                        av[:, sc, :], vh_aug[:, t, h, :],
                        pt[:, t, sc * SCW:(sc + 1) * SCW],
                        start=(t == 0), stop=(t == NT - 1))

            def av_norm(h, av, oT, sc=None):
                par = 64 * (h % 2)
                j = h // 2
                if sc is not None:
                    recip = sm.tile([1, SCW], F32, tag="recip")
                    nc.vector.reciprocal(recip, av[DV:DV + 1, sc, :])
                    rbc = sm.tile([DV, SCW], F32, tag="rbc")
                    nc.gpsimd.partition_broadcast(rbc, recip)
                    nc.vector.tensor_mul(
                        oT[par:par + 64, j, sc * SCW:(sc + 1) * SCW],
                        av[0:DV, sc, :], rbc[:])
                    return
                recip = sm.tile([1, SQ], F32, tag="recip2")
                nc.vector.reciprocal(
                    recip, av[DV:DV + 1].rearrange("p a b -> p (a b)"))
                rbc = sm.tile([DV, SQ], F32, tag="rbc2")
                nc.gpsimd.partition_broadcast(rbc, recip)
                nc.vector.tensor_mul(
                    oT[par:par + 64, j, :],
                    av[0:DV].rearrange("p a b -> p (a b)"), rbc[:])

            pre_q = []   # data-producing units: popped before scores(h+1)
            post_q = []  # consumer-only units: popped after scores(h+1)

            def pop_units(q, n):
                for _ in range(min(n, len(q))):
                    q.pop(0)()

            def attention(b, sp_first, next_scores=None):
                oT = otp.tile([P, H // 2, SQ], F32R, tag="ot")
                sp_cur = sp_first
                ret = None
                def sc_chunk(hh, t0, t1, sp):
                    if hh is None:
                        return
                    if hh == "next":
                        # guard: head-pair 0 of b+1 must be projected first
                        while m_done.get(b + 1, -1) < 0 and pre_q:
                            pop_units(pre_q, 1)
                        ret2 = next_scores(t0, t1, sp)
                        return ret2
                    while m_done.get(b, -1) < hh // 2 and pre_q:
                        pop_units(pre_q, 1)
                    scores(hh, preps[b][2], preps[b][3], t0=t0, t1=t1, sp=sp)
                    return sp

                for h in range(H):
                    pt = ptp.tile([P, NT, SQ], BF16, tag="pt")
                    exp_head(h, sp_cur, pt)
                    av = av_ps.tile([DV + 1, SCH, SCW], F32, tag="av")
                    if h + 1 < H:
                        nh, nsp = h + 1, []
                        sp_cur = nsp
                    elif next_scores is not None:
                        nh, nsp = "next", []
                        ret = nsp
                    else:
                        nh, nsp = None, None
                    pop_units(pre_q, 2 if h == 0 else 1)
                    sc_chunk(nh, 0, 2, nsp)
                    if h >= 2:
                        pop_units(post_q, 1)
                    sc_chunk(nh, 2, 4, nsp)
                    if b > 0:
                        while b not in vh_done and pre_q:
                            pop_units(pre_q, 1)
                    av_half(h, pt, preps[b][4], av, 0)
                    if nh is None:
                        # tail: normalize sc0 as soon as its AV half is done
                        av_norm(h, av, oT, 0)
                        av_half(h, pt, preps[b][4], av, 1)
                        av_norm(h, av, oT, 1)
                    else:
                        sc_chunk(nh, 4, 6, nsp)
                        av_half(h, pt, preps[b][4], av, 1)
                        av_norm(h, av, oT)
                        sc_chunk(nh, 6, 8, nsp)
                return oT, ret

            def out_proj_sc(b, oT, ct, sc):
                """p[c, s] = WpT.T @ oT for one (ct, sc) chunk + stats."""
                pj = mm_ps.tile([P, SCW], F32, tag="mm")
                for g in range(H // 2):
                    nc.tensor.matmul(
                        pj[:],
                        wp_sb[:, g, ct * P:(ct + 1) * P],
                        oT[:, g, sc * SCW:(sc + 1) * SCW],
                        start=(g == 0), stop=(g == H // 2 - 1))
                nc.vector.tensor_scalar(
                    p_sb[:, ct, b, sc * SCW:(sc + 1) * SCW], pj[:],
                    1.0, 0.0, mybir.AluOpType.mult, mybir.AluOpType.add,
                    accum_out=s_parts[:, 4 * ct + 2 * b + sc:
                                      4 * ct + 2 * b + sc + 1])
                psl = p_sb[:, ct, b, sc * SCW:(sc + 1) * SCW]
                nc.vector.scalar_tensor_tensor(
                    psq_scratch[:, 0:SCW], psl, 1.0, psl,
                    mybir.AluOpType.mult, mybir.AluOpType.mult,
                    accum_out=sq_parts[:, 4 * ct + 2 * b + sc:
                                       4 * ct + 2 * b + sc + 1])

            def out_proj_ct(b, oT, ct):
                for sc in range(SCH):
                    out_proj_sc(b, oT, ct, sc)

            # ---------------- emission ----------------
            psq_scratch = sm.tile([P, SQ], BF16, tag="psq")
            sq_parts = fin.tile([P, 4 * BL], F32, tag="sqp")
            s_parts = fin.tile([P, 4 * BL], F32, tag="sp_")

            prep_start(0)
            sp_cur = scores(0, preps[0][2], preps[0][3])
            nc.sync.dma_start(out=wv_sb, in_=wv_d[:])
            nc.sync.dma_start(out=wp_sb, in_=wp_d[:])
            nc.sync.dma_start(out=gb_sb, in_=gb_d[:])
            vh_alloc(0)
            vproj(0, 0, 8)
            pre_q.extend(prep_units(0, with_start=False))

            oTs = {}

            def make_next_scores(b):
                def f(t0, t1, sp):
                    return scores(0, preps[b][2], preps[b][3], t0=t0, t1=t1,
                                  sp=sp)
                return f

            for b in range(BL):
                last = b + 1 >= BL
                if not last:
                    pre_q.extend(prep_units(b + 1, with_start=True))
                if b > 0:
                    for sc in range(SCH):
                        for ct in range(2):
                            post_q.append(
                                lambda ct=ct, sc=sc, bb=b - 1: out_proj_sc(
                                    bb, oTs[bb], ct, sc))
                oT, sp_cur = attention(
                    b, sp_cur,
                    next_scores=None if last else make_next_scores(b + 1))
                oTs[b] = oT
            for sc in range(SCH):
                for ct in range(2):
                    out_proj_sc(BL - 1, oTs[BL - 1], ct, sc)

            # ---- BN statistics ----
            stats = fin.tile([P, 2, 2], F32, tag="stats")  # [c, ct, {s, s2}]
            nc.vector.tensor_reduce(
                stats[:, :, 0], s_parts.rearrange("p (c x) -> p c x", c=2),
                mybir.AxisListType.X, mybir.AluOpType.add)
            nc.vector.tensor_reduce(
                stats[:, :, 1], sq_parts.rearrange("p (c x) -> p c x", c=2),
                mybir.AxisListType.X, mybir.AluOpType.add)

            # ---- all-reduce stats across cores ----
            if with_collective:
                ar_in = dram.tile([P, 4], F32)
                ar_out = dram.tile([P, 4], F32)
                nc.sync.dma_start(out=ar_in[:],
                                  in_=stats.rearrange("p a b -> p (a b)"))
                nc.gpsimd.collective_compute(
                    "AllReduce", mybir.AluOpType.add,
                    replica_groups=[list(range(n_cores))],
                    ins=[ar_in.opt()], outs=[ar_out.opt()])
                g_sb = fin.tile([P, 2, 2], F32, tag="g")
                nc.sync.dma_start(out=g_sb.rearrange("p a b -> p (a b)"),
                                  in_=ar_out[:])
            else:
                g_sb = stats

            # ---- finalize BN scale/bias (fused [P, 2] ops over ct) ----
            n_total = float(B * SQ) if with_collective else float(BL * SQ)
            eps_t = fin.tile([P, 1], F32, tag="eps")
            nc.vector.memset(eps_t, BN_EPS)
            a_ap = fin.tile([P, 2], F32, tag="a")
            b_ap = fin.tile([P, 2], F32, tag="b")
            mean2 = fin.tile([P, 2], F32, tag="mean2")
            msq2 = fin.tile([P, 2], F32, tag="msq2")
            var2 = fin.tile([P, 2], F32, tag="var2")
            sd2 = fin.tile([P, 2], F32, tag="sd2")
            rstd2 = fin.tile([P, 2], F32, tag="rstd2")
            bm2 = fin.tile([P, 2], F32, tag="bm2")
            nc.vector.tensor_scalar(mean2, g_sb[:, :, 0], 1.0 / n_total, None,
                                    mybir.AluOpType.mult)
            nc.vector.tensor_scalar(msq2, g_sb[:, :, 1], 1.0 / n_total, None,
                                    mybir.AluOpType.mult)
            nc.vector.tensor_mul(var2, mean2, mean2)
            nc.vector.tensor_sub(var2, msq2, var2)
            nc.scalar.activation(out=sd2, in_=var2,
                                 func=mybir.ActivationFunctionType.Ln,
                                 bias=eps_t[:, 0:1])
            nc.scalar.activation(out=rstd2, in_=sd2,
                                 func=mybir.ActivationFunctionType.Exp,
                                 scale=-0.5)
            nc.vector.tensor_mul(a_ap, rstd2, gb_sb[:, :, 0])
            nc.vector.tensor_mul(bm2, mean2, a_ap)
            nc.vector.tensor_sub(b_ap, gb_sb[:, :, 1], bm2)

            # ---- BN apply + LeakyReLU (ACT) + store ([c, s]; host transposes)
            y_all = yyp.tile([P, 2, BL, SQ], BF16, tag="yall")
            for b in range(BL):
                for ct in range(2):
                    nc.scalar.activation(
                        out=y_all[:, ct, b, :],
                        in_=p_sb[:, ct, b, :],
                        func=mybir.ActivationFunctionType.Prelu,
                        scale=a_ap[:, ct:ct + 1], bias=b_ap[:, ct:ct + 1],
                        alpha=NEG_SLOPE)
                    nc.sync.dma_start(
                        out=y_d[b, ct], in_=y_all[:, ct, b, :])

    nc.compile()
    return nc


def prep_weights(Wq, Wk, Wv, Wp, gamma, beta):
    wq = np.ascontiguousarray(
        Wq.transpose(2, 0, 1).reshape(2, P, H, DK)
        .transpose(1, 0, 2, 3)).astype(np.float32)
    wk = np.ascontiguousarray(
        Wk.transpose(2, 0, 1).reshape(2, P, H, DK)
        .transpose(1, 0, 2, 3)).astype(np.float32)
    wv = np.ascontiguousarray(
        Wv.transpose(2, 0, 1).reshape(2, P, H * DV)
        .transpose(1, 0, 2)).astype(np.float32)
    # wp: [128 (he within group), group, c] with he = h*64+e head-major
    wpT = Wp.T.reshape(H // 2, P, C)  # [g, he%128, c]
    wp = np.ascontiguousarray(wpT.transpose(1, 0, 2)).astype(np.float32)
    # gamma/beta in [c%128, ct, {gamma,beta}]
    gb = np.stack([gamma.reshape(2, P), beta.reshape(2, P)], axis=-1)
    gb = np.ascontiguousarray(gb.transpose(1, 0, 2)).astype(np.float32)
    return wq, wk, wv, wp, gb


_NC_CACHE = {}


def kernel(x, q, Wq, Wk, Wv, Wp, gamma, beta):
    x = np.asarray(x, dtype=np.float32)
    q = np.asarray(q, dtype=np.float32)
    wq, wk, wv, wp, gb = prep_weights(
        np.asarray(Wq, np.float32), np.asarray(Wk, np.float32),
        np.asarray(Wv, np.float32), np.asarray(Wp, np.float32),
        np.asarray(gamma, np.float32), np.asarray(beta, np.float32))

    if "nc" not in _NC_CACHE:
        _NC_CACHE["nc"] = build_kernel()
    nc = _NC_CACHE["nc"]

    # host-side transpose: [BL, S, C] -> [BL, 2, 128, S]
    def t_in(a):
        return np.ascontiguousarray(
            a.transpose(0, 2, 1).reshape(a.shape[0], 2, P, a.shape[1]))

    in_maps = []
    for i in range(N_CORES):
        in_maps.append({
            "qt": t_in(q[i * BL:(i + 1) * BL]).astype(np.float32),
            "xt": t_in(x[i * BL:(i + 1) * BL]).astype(np.float32),
            "wq": wq, "wk": wk, "wv": wv, "wp": wp, "gb": gb,
        })
    res = run_bass_kernel_spmd(nc, in_maps, list(range(N_CORES)))
    outs = []
    for i in range(N_CORES):
        y = np.asarray(res.results[i]["y"]).astype(np.float32)
        y = y.reshape(BL, 2, P, SQ).transpose(0, 3, 1, 2).reshape(BL, SQ, C)
        outs.append(y)
    return np.concatenate(outs, axis=0)


# revision 8
# speedup vs baseline: 1.0148x; 1.0148x over previous
"""MultiHeadCrossAttention Trainium2 kernel (8-core data-parallel).

Shapes (hardcoded): B=16, SQ=SE=1024, C_IN=C_ENC=256, DK=DV=64, H=8.
Sharding: batch across 8 cores (2 batches/core). 189.8us cost-model time
per core vs 274.5us for the v1 baseline.

Design notes:
- q/x are pre-transposed to [C, S] on the host (part of the sharding
  prep), eliminating all PE transposes and their PSUM evacuations.
- ScalarE (ACT) runs ONLY exp -- it is the critical engine at ~133us
  busy (16.8M softmax elements / 128 lanes / 1.2GHz). All other
  elementwise work is kept on DVE/Pool. All ACT funcs used (Exp, Ln,
  Prelu) live in one PWP table set to avoid mid-kernel table swaps.
- Attention inner loop is software-pipelined by emission order:
  exp(h) | scores(h+1) in 2-tile chunks interleaved with the two AV
  half-accumulations of head h, so PE stays just ahead of ACT under the
  2-buffer PSUM rotation. Background work (next batch's projections,
  previous batch's output projection) is queued as ~1.2us units popped
  one per head iteration; guards force-drain producer units before
  their consumers are emitted.
- Softmax denominator comes free as a ones-column in V (row 64 of the
  AV accumulation); normalization is reciprocal (DVE) + partition
  broadcast (Pool) + multiply (DVE).
- Output projection is computed in [c, s] layout (WpT as stationary) so
  BatchNorm statistics reduce along the free axis: sum(p) rides the
  PSUM-evacuation tensor_scalar's accum_out, sum(p^2) is one
  scalar_tensor_tensor with accum_out per (ct, b, sc) chunk.
- BN scale/bias finalize uses rstd = exp(-0.5*ln(var+eps)) (same ACT
  table set); apply+LeakyReLU is a single Prelu activation per (b, ct)
  with per-partition scale/bias, stored straight to DRAM in [c, s]
  layout -- the host does the final [c, s] -> [s, c] transpose.
- Outputs are bf16 (rounded); inputs stay fp32/f32r so scores keep full
  precision. Hardware-validated rel err ~5e-3 (gate 2e-2).

Hardware pitfalls encountered (real constraints, not in the cost model):
- GPSIMD cannot touch PSUM; scalar_tensor_tensor is DVE-only.
- Interleaving two open PSUM matmul accumulation groups faults the PE.
- fp32r operands must be produced as f32r (rounded) by their writer.
- TensorScalar with accum_out requires both ops; only one non-scalar
  PSUM input per DVE instruction.
- fp8 (e4m3) scores fail the 2e-2 gate (~6.5e-2) -- exp amplifies the
  ~5% quantization noise; DoubleRow is therefore not usable here.
"""
import sys

sys.path.insert(0, "/opt/trn_rl_repo")

import numpy as np

import concourse.bacc as bacc
import concourse.tile as tile
from concourse import mybir
from concourse.bass_utils import run_bass_kernel_spmd

F32 = mybir.dt.float32
F32R = mybir.dt.float32r
BF16 = mybir.dt.bfloat16

B, SQ, SE = 16, 1024, 1024
C, DK, DV, H = 256, 64, 64, 8
BN_EPS = 1e-5
NEG_SLOPE = 0.01
N_CORES = 8
BL = B // N_CORES
P = 128
NT = SE // P   # 8 key tiles
NST = SQ // P  # 8 query tiles
SCH = 2        # s-chunks of 512 per head
SCW = SQ // SCH


def build_kernel(n_cores=N_CORES, with_collective=True):
    nc = bacc.Bacc("TRN2", target_bir_lowering=False, debug=False,
                   num_devices=n_cores)

    qt_d = nc.declare_dram_parameter("qt", [BL, 2, P, SQ], F32R, isOutput=False)
    xt_d = nc.declare_dram_parameter("xt", [BL, 2, P, SE], F32R, isOutput=False)
    wq_d = nc.declare_dram_parameter("wq", [P, 2, H, DK], F32R, isOutput=False)
    wk_d = nc.declare_dram_parameter("wk", [P, 2, H, DK], F32R, isOutput=False)
    wv_d = nc.declare_dram_parameter("wv", [P, 2, H * DV], F32R, isOutput=False)
    wp_d = nc.declare_dram_parameter("wp", [P, H // 2, C], F32R, isOutput=False)
    gb_d = nc.declare_dram_parameter("gb", [P, 2, 2], F32, isOutput=False)
    y_d = nc.declare_dram_parameter("y", [BL, 2, P, SQ], BF16, isOutput=True)

    with tile.TileContext(nc) as tc:
        with (
            tc.tile_pool(name="const", bufs=1) as const,
            tc.tile_pool(name="qx", bufs=2) as qxp,       # qT/xT inputs
            tc.tile_pool(name="qk", bufs=2) as qkp,       # qhT/khT projections
            tc.tile_pool(name="vh", bufs=2) as vhp,       # vh_aug values
            tc.tile_pool(name="pt", bufs=2) as ptp,       # exp(scores)
            tc.tile_pool(name="ot", bufs=2) as otp,       # normalized attn out
            tc.tile_pool(name="pp", bufs=1) as ppp,       # projected p (both b)
            tc.tile_pool(name="sm", bufs=2) as sm,        # small scratch
            tc.tile_pool(name="yy", bufs=2) as yyp,       # y staging
            tc.tile_pool(name="fin", bufs=1) as fin,
            tc.tile_pool(name="sp_ps", bufs=2, space="PSUM") as sp_ps,   # 2x4KB
            tc.tile_pool(name="av_ps", bufs=1, space="PSUM") as av_ps,   # 1x4KB
            tc.tile_pool(name="mm_ps", bufs=2, space="PSUM") as mm_ps,   # 2x2KB
            tc.tile_pool(name="dram", bufs=1, space="DRAM") as dram,
        ):
            wq_sb = const.tile([P, 2, H, DK], F32R, tag="wq")
            wk_sb = const.tile([P, 2, H, DK], F32R, tag="wk")
            wv_sb = const.tile([P, 2, H * DV], F32R, tag="wv")
            wp_sb = const.tile([P, H // 2, C], F32R, tag="wp")
            gb_sb = const.tile([P, 2, 2], F32, tag="gb")
            # wk issued first, then xT (in prep_start), then wq/qT;
            # wv/wp/gb issued after the input loads.
            nc.sync.dma_start(out=wk_sb, in_=wk_d[:])

            # projected p for both batches, [c, ct, b, s] layout
            p_sb = ppp.tile([P, 2, BL, SQ], BF16, tag="p")

            def proj_slab(w_sb, src, dst, m, sc0=0, sc1=SCH):
                for sc in range(sc0, sc1):
                    pj = mm_ps.tile([P, SCW], F32, tag="mm")
                    for k in range(2):
                        nc.tensor.matmul(
                            pj[:],
                            w_sb[:, k, 2 * m:2 * m + 2, :],
                            src[:, k, sc * SCW:(sc + 1) * SCW],
                            start=(k == 0), stop=(k == 1))
                    nc.vector.tensor_copy(
                        dst[:, m, sc * SCW:(sc + 1) * SCW], pj[:])

            preps = {}
            vh_done = {}
            m_done = {}

            def prep_start(b):
                """Load qT/xT; project head-pair 0 only (unblocks scores(0))."""
                xT = qxp.tile([P, 2, SE], F32R, tag="qx")
                qT = qxp.tile([P, 2, SQ], F32R, tag="qx")
                for k in range(2):
                    nc.sync.dma_start(out=xT[:, k, :], in_=xt_d[b, k])
                if b == 0:
                    nc.sync.dma_start(out=wq_sb, in_=wq_d[:])
                for k in range(2):
                    nc.sync.dma_start(out=qT[:, k, :], in_=qt_d[b, k])
                qhT = qkp.tile([P, H // 2, SQ], F32R, tag="qk")
                khT = qkp.tile([P, H // 2, SE], F32R, tag="qk")
                preps[b] = (qT, xT, qhT, khT, None)
                m_done[b] = -1
                proj_slab(wk_sb, xT, khT, 0)
                if b == 0:
                    proj_slab(wq_sb, qT, qhT, 0)
                    m_done[b] = 0

            def vh_alloc(b):
                qT, xT, qhT, khT, _ = preps[b]
                vh_aug = vhp.tile([P, NT, H, DV + 1], BF16, tag="vh")
                nc.vector.memset(vh_aug[:, :, :, DV:DV + 1], 1.0)
                preps[b] = (qT, xT, qhT, khT, vh_aug)

            def vproj(b, t0, t1):
                qT, xT, qhT, khT, vh_aug = preps[b]
                for t in range(t0, t1):
                    pj = mm_ps.tile([P, H * DV], F32, tag="mm")
                    for k in range(2):
                        nc.tensor.matmul(
                            pj[:], xT[:, k, t * P:(t + 1) * P], wv_sb[:, k, :],
                            start=(k == 0), stop=(k == 1))
                    nc.vector.tensor_copy(
                        vh_aug[:, t, :, 0:DV],
                        pj.rearrange("p (h e) -> p h e", h=H))

            def prep_units(b, with_start):
                """Work units (~1.2us PE each) finishing what prep_start began."""
                units = []
                def qslab(m):
                    proj_slab(wq_sb, preps[b][0], preps[b][2], m)
                    m_done[b] = m

                if with_start:
                    units.append(lambda: prep_start(b))
                    units.append(lambda: qslab(0))
                    units.append(lambda: (vh_alloc(b), vproj(b, 0, 4)))
                    units.append(lambda: (vproj(b, 4, 8),
                                          vh_done.__setitem__(b, True)))
                for m in range(1, H // 2):
                    units.append(lambda m=m: proj_slab(
                        wk_sb, preps[b][1], preps[b][3], m))
                    units.append(lambda m=m: qslab(m))
                return units

            def scores(h, qhT, khT, t0=0, t1=NT, sp=None):
                """Raw attention scores for head h: NT x [key 128, 2, 512] PSUM."""
                par = 64 * (h % 2)
                j = h // 2
                if sp is None:
                    sp = []
                for t in range(t0, t1):
                    spt = sp_ps.tile([P, SCH, SCW], F32, tag="sp")
                    for sc in range(SCH):
                        nc.tensor.matmul(
                            spt[:, sc, :],
                            khT[par:par + 64, j, t * P:(t + 1) * P],
                            qhT[par:par + 64, j, sc * SCW:(sc + 1) * SCW],
                            start=True, stop=True)
                    sp.append(spt)
                return sp

            def exp_head(h, sp, pt):
                for t in range(NT):
                    nc.scalar.activation(
                        out=pt[:, t, :],
                        in_=sp[t].rearrange("p a b -> p (a b)"),
                        func=mybir.ActivationFunctionType.Exp,
                        scale=1.0 / np.sqrt(DK).item())

            def av_half(h, pt, vh_aug, av, sc):
                for t in range(NT):
                    nc.tensor.matmul(
                        av[:, sc, :], vh_aug[:, t, h, :],
                        pt[:, t, sc * SCW:(sc + 1) * SCW],
                        start=(t == 0), stop=(t == NT - 1))

            def av_norm(h, av, oT, sc=None):
                par = 64 * (h % 2)
                j = h // 2
                if sc is not None:
                    recip = sm.tile([1, SCW], F32, tag="recip")
                    nc.vector.reciprocal(recip, av[DV:DV + 1, sc, :])
                    rbc = sm.tile([DV, SCW], F32, tag="rbc")
                    nc.gpsimd.partition_broadcast(rbc, recip)
                    nc.vector.tensor_mul(
                        oT[par:par + 64, j, sc * SCW:(sc + 1) * SCW],
                        av[0:DV, sc, :], rbc[:])
                    return
                recip = sm.tile([1, SQ], F32, tag="recip2")
                nc.vector.reciprocal(
                    recip, av[DV:DV + 1].rearrange("p a b -> p (a b)"))
                rbc = sm.tile([DV, SQ], F32, tag="rbc2")
                nc.gpsimd.partition_broadcast(rbc, recip)
                nc.vector.tensor_mul(
                    oT[par:par + 64, j, :],
                    av[0:DV].rearrange("p a b -> p (a b)"), rbc[:])

            pre_q = []   # data-producing units: popped before scores(h+1)
            post_q = []  # consumer-only units: popped after scores(h+1)

            def pop_units(q, n):
                for _ in range(min(n, len(q))):
                    q.pop(0)()

            def attention(b, sp_first, next_scores=None):
                oT = otp.tile([P, H // 2, SQ], F32R, tag="ot")
                sp_cur = sp_first
                ret = None
                def sc_chunk(hh, t0, t1, sp):
                    if hh is None:
                        return
                    if hh == "next":
                        # guard: head-pair 0 of b+1 must be projected first
                        while m_done.get(b + 1, -1) < 0 and pre_q:
                            pop_units(pre_q, 1)
                        ret2 = next_scores(t0, t1, sp)
                        return ret2
                    while m_done.get(b, -1) < hh // 2 and pre_q:
                        pop_units(pre_q, 1)
                    scores(hh, preps[b][2], preps[b][3], t0=t0, t1=t1, sp=sp)
                    return sp

                for h in range(H):
                    pt = ptp.tile([P, NT, SQ], BF16, tag="pt")
                    exp_head(h, sp_cur, pt)
                    av = av_ps.tile([DV + 1, SCH, SCW], F32, tag="av")
                    if h + 1 < H:
                        nh, nsp = h + 1, []
                        sp_cur = nsp
                    elif next_scores is not None:
                        nh, nsp = "next", []
                        ret = nsp
                    else:
                        nh, nsp = None, None
                    pop_units(pre_q, 2 if h == 0 else 1)
                    sc_chunk(nh, 0, 2, nsp)
                    if h >= 2:
                        pop_units(post_q, 1)
                    sc_chunk(nh, 2, 4, nsp)
                    if b > 0:
                        while b not in vh_done and pre_q:
                            pop_units(pre_q, 1)
                    av_half(h, pt, preps[b][4], av, 0)
                    if nh is None:
                        # tail: normalize sc0 as soon as its AV half is done
                        av_norm(h, av, oT, 0)
                        av_half(h, pt, preps[b][4], av, 1)
                        av_norm(h, av, oT, 1)
                    else:
                        sc_chunk(nh, 4, 6, nsp)
                        av_half(h, pt, preps[b][4], av, 1)
                        av_norm(h, av, oT)
                        sc_chunk(nh, 6, 8, nsp)
                return oT, ret

            def out_proj_sc(b, oT, ct, sc):
                """p[c, s] = WpT.T @ oT for one (ct, sc) chunk + stats."""
                pj = mm_ps.tile([P, SCW], F32, tag="mm")
                for g in range(H // 2):
                    nc.tensor.matmul(
                        pj[:],
                        wp_sb[:, g, ct * P:(ct + 1) * P],
                        oT[:, g, sc * SCW:(sc + 1) * SCW],
                        start=(g == 0), stop=(g == H // 2 - 1))
                nc.vector.tensor_scalar(
                    p_sb[:, ct, b, sc * SCW:(sc + 1) * SCW], pj[:],
                    1.0, 0.0, mybir.AluOpType.mult, mybir.AluOpType.add,
                    accum_out=s_parts[:, 4 * ct + 2 * b + sc:
                                      4 * ct + 2 * b + sc + 1])
                psl = p_sb[:, ct, b, sc * SCW:(sc + 1) * SCW]
                nc.vector.scalar_tensor_tensor(
                    psq_scratch[:, 0:SCW], psl, 1.0, psl,
                    mybir.AluOpType.mult, mybir.AluOpType.mult,
                    accum_out=sq_parts[:, 4 * ct + 2 * b + sc:
                                       4 * ct + 2 * b + sc + 1])


            # ---------------- emission ----------------
            psq_scratch = sm.tile([P, SQ], BF16, tag="psq")
            sq_parts = fin.tile([P, 4 * BL], F32, tag="sqp")
            s_parts = fin.tile([P, 4 * BL], F32, tag="sp_")

            prep_start(0)
            sp_cur = scores(0, preps[0][2], preps[0][3])
            nc.sync.dma_start(out=wv_sb, in_=wv_d[:])
            nc.sync.dma_start(out=wp_sb, in_=wp_d[:])
            nc.sync.dma_start(out=gb_sb, in_=gb_d[:])
            vh_alloc(0)
            vproj(0, 0, 8)
            pre_q.extend(prep_units(0, with_start=False))

            oTs = {}

            def make_next_scores(b):
                def f(t0, t1, sp):
                    return scores(0, preps[b][2], preps[b][3], t0=t0, t1=t1,
                                  sp=sp)
                return f

            for b in range(BL):
                last = b + 1 >= BL
                if not last:
                    pre_q.extend(prep_units(b + 1, with_start=True))
                if b > 0:
                    for sc in range(SCH):
                        for ct in range(2):
                            post_q.append(
                                lambda ct=ct, sc=sc, bb=b - 1: out_proj_sc(
                                    bb, oTs[bb], ct, sc))
                oT, sp_cur = attention(
                    b, sp_cur,
                    next_scores=None if last else make_next_scores(b + 1))
                oTs[b] = oT
            for sc in range(SCH):
                for ct in range(2):
                    out_proj_sc(BL - 1, oTs[BL - 1], ct, sc)

            # ---- BN statistics ----
            stats = fin.tile([P, 2, 2], F32, tag="stats")  # [c, ct, {s, s2}]
            nc.vector.tensor_reduce(
                stats[:, :, 0], s_parts.rearrange("p (c x) -> p c x", c=2),
                mybir.AxisListType.X, mybir.AluOpType.add)
            nc.vector.tensor_reduce(
                stats[:, :, 1], sq_parts.rearrange("p (c x) -> p c x", c=2),
                mybir.AxisListType.X, mybir.AluOpType.add)

            # ---- all-reduce stats across cores ----
            if with_collective:
                ar_in = dram.tile([P, 4], F32)
                ar_out = dram.tile([P, 4], F32)
                nc.sync.dma_start(out=ar_in[:],
                                  in_=stats.rearrange("p a b -> p (a b)"))
                nc.gpsimd.collective_compute(
                    "AllReduce", mybir.AluOpType.add,
                    replica_groups=[list(range(n_cores))],
                    ins=[ar_in.opt()], outs=[ar_out.opt()])
                g_sb = fin.tile([P, 2, 2], F32, tag="g")
                nc.sync.dma_start(out=g_sb.rearrange("p a b -> p (a b)"),
                                  in_=ar_out[:])
            else:
                g_sb = stats

            # ---- finalize BN scale/bias (fused [P, 2] ops over ct) ----
            n_total = float(B * SQ) if with_collective else float(BL * SQ)
            eps_t = fin.tile([P, 1], F32, tag="eps")
            nc.vector.memset(eps_t, BN_EPS)
            a_ap = fin.tile([P, 2], F32, tag="a")
            b_ap = fin.tile([P, 2], F32, tag="b")
            mean2 = fin.tile([P, 2], F32, tag="mean2")
            msq2 = fin.tile([P, 2], F32, tag="msq2")
            var2 = fin.tile([P, 2], F32, tag="var2")
            sd2 = fin.tile([P, 2], F32, tag="sd2")
            rstd2 = fin.tile([P, 2], F32, tag="rstd2")
            bm2 = fin.tile([P, 2], F32, tag="bm2")
            nc.vector.tensor_scalar(mean2, g_sb[:, :, 0], 1.0 / n_total, None,
                                    mybir.AluOpType.mult)
            nc.vector.tensor_scalar(msq2, g_sb[:, :, 1], 1.0 / n_total, None,
                                    mybir.AluOpType.mult)
            nc.vector.tensor_mul(var2, mean2, mean2)
            nc.vector.tensor_sub(var2, msq2, var2)
            nc.scalar.activation(out=sd2, in_=var2,
                                 func=mybir.ActivationFunctionType.Ln,
                                 bias=eps_t[:, 0:1])
            nc.scalar.activation(out=rstd2, in_=sd2,
                                 func=mybir.ActivationFunctionType.Exp,
                                 scale=-0.5)
            nc.vector.tensor_mul(a_ap, rstd2, gb_sb[:, :, 0])
            nc.vector.tensor_mul(bm2, mean2, a_ap)
            nc.vector.tensor_sub(b_ap, gb_sb[:, :, 1], bm2)

            # ---- BN apply + LeakyReLU (ACT) + store ([c, s]; host transposes)
            y_all = yyp.tile([P, 2, BL, SQ], BF16, tag="yall")
            for b in range(BL):
                for ct in range(2):
                    nc.scalar.activation(
                        out=y_all[:, ct, b, :],
                        in_=p_sb[:, ct, b, :],
                        func=mybir.ActivationFunctionType.Prelu,
                        scale=a_ap[:, ct:ct + 1], bias=b_ap[:, ct:ct + 1],
                        alpha=NEG_SLOPE)
                    nc.sync.dma_start(
                        out=y_d[b, ct], in_=y_all[:, ct, b, :])

    nc.compile()
    return nc


def prep_weights(Wq, Wk, Wv, Wp, gamma, beta):
    wq = np.ascontiguousarray(
        Wq.transpose(2, 0, 1).reshape(2, P, H, DK)
        .transpose(1, 0, 2, 3)).astype(np.float32)
    wk = np.ascontiguousarray(
        Wk.transpose(2, 0, 1).reshape(2, P, H, DK)
        .transpose(1, 0, 2, 3)).astype(np.float32)
    wv = np.ascontiguousarray(
        Wv.transpose(2, 0, 1).reshape(2, P, H * DV)
        .transpose(1, 0, 2)).astype(np.float32)
    # wp: [128 (he within group), group, c] with he = h*64+e head-major
    wpT = Wp.T.reshape(H // 2, P, C)  # [g, he%128, c]
    wp = np.ascontiguousarray(wpT.transpose(1, 0, 2)).astype(np.float32)
    # gamma/beta in [c%128, ct, {gamma,beta}]
    gb = np.stack([gamma.reshape(2, P), beta.reshape(2, P)], axis=-1)
    gb = np.ascontiguousarray(gb.transpose(1, 0, 2)).astype(np.float32)
    return wq, wk, wv, wp, gb


_NC_CACHE = {}


def kernel(x, q, Wq, Wk, Wv, Wp, gamma, beta):
    x = np.asarray(x, dtype=np.float32)
    q = np.asarray(q, dtype=np.float32)
    wq, wk, wv, wp, gb = prep_weights(
        np.asarray(Wq, np.float32), np.asarray(Wk, np.float32),
        np.asarray(Wv, np.float32), np.asarray(Wp, np.float32),
        np.asarray(gamma, np.float32), np.asarray(beta, np.float32))

    if "nc" not in _NC_CACHE:
        _NC_CACHE["nc"] = build_kernel()
    nc = _NC_CACHE["nc"]

    # host-side transpose: [BL, S, C] -> [BL, 2, 128, S]
    def t_in(a):
        return np.ascontiguousarray(
            a.transpose(0, 2, 1).reshape(a.shape[0], 2, P, a.shape[1]))

    in_maps = []
    for i in range(N_CORES):
        in_maps.append({
            "qt": t_in(q[i * BL:(i + 1) * BL]).astype(np.float32),
            "xt": t_in(x[i * BL:(i + 1) * BL]).astype(np.float32),
            "wq": wq, "wk": wk, "wv": wv, "wp": wp, "gb": gb,
        })
    res = run_bass_kernel_spmd(nc, in_maps, list(range(N_CORES)))
    outs = []
    for i in range(N_CORES):
        y = np.asarray(res.results[i]["y"]).astype(np.float32)
        y = y.reshape(BL, 2, P, SQ).transpose(0, 3, 1, 2).reshape(BL, SQ, C)
        outs.append(y)
    return np.concatenate(outs, axis=0)
